# revision 48
# baseline (speedup 1.0000x reference)
"""DFL loss (nn_DFLLoss) Trainium2 Bass kernel — 8-core data parallel.

reference computes, per (batch, pixel, coord j in 0..3):
    rl[b, hw, j, k] = reg_logits[b, j*8+k, hw]          (k in 0..7 bins)
    t = clip(targets, 0, 6.9999); l = floor(t); u = l+1
    per = w_l * (lse - rl[l]) + w_u * (lse - rl[u]),  lse = logsumexp_k rl
    loss = sum(per * pos_mask) / (max(sum(pos_mask), 1) * 4)

Decomposition: masked_total = sum(mask*lse) - sum_k relu(1-|t''-k|)*rl[k]
with t'' = t + 100*mask (masked-out pixels give hat weight exactly 0).

Engine plan (per core, 4 batches):
- Logits are uploaded TWICE in fp8e4 (together = one bf16 copy's bytes):
  * px-major [blk, j, 8ch, px] for the interp side: the fused hat custom
    DVE op per (batch, j) unit — DVE is the bottleneck at 16x1667ns.
  * ch-major [(ch,strip) part, px] for the lse side: batched exp on
    ScalarE, then bin sums via PE matmuls with the exp tile as the
    STATIONARY operand and a tiny fixed [128,16] selector as the moving
    operand — out free size is 16, so all 200 matmuls cost ~1us of PE
    and the sums land PIXEL-MAJOR [128px, (g,j,s)] in PSUM. ln on the
    dense psum tile, mask-multiply on GpSimd, accumulate on GpSimd.
- ScalarE: exp (4x[128,6400] in halves) + ln; ~26us. DVE: hats only,
  26.7us. GpSimd: t-prep + masked-lse + npos ~19us. DMA ~21us.
The act-table patch keeps Exp and Ln in one table set (no reloads).
"""

import threading

import numpy as np

BINS = 8
B, C, H, W = 32, 32, 160, 160
HW = H * W  # 25600
NCORES = 8
BPC = B // NCORES  # 4 batches per core
PX = HW // 128  # 200 pixels per partition per batch (px-major side)
NJ = 4
NSTRIP = 4  # ch-major strips per batch: partition p = ch*4 + s
SPX = HW // NSTRIP  # 6400 pixels per strip
NG = SPX // 128  # 50 pixel-groups of 128 per strip
NACC = 32  # acc columns: [0:16]+[16] interp, [17:25] masked lse per
#            (batch, half), [25:29] 100*npos per batch, rest zero


_lock = threading.Lock()
_cache: dict = {}


def _register_hat_op():
    """Register the fused hat*logit+reduce custom DVE op (idempotent)."""
    from operator import add as _operator_add

    import concourse.dve_ops as dve_ops
    from concourse.dve_spec import (
        C0,
        C1,
        PageIdx,
        Spec,
        Src0,
        Src1,
        Zero,
        One,
        lower,
        maxx,
        relu,
    )
    from concourse.dve_uop import DveOpSpec

    name = "HAT_MUL_ACC_DFL"
    if name in dve_ops._SUB_OPCODE_FOR_NAME:
        for op in dve_ops.OPS:
            if op.name == name:
                return op

    _pg = PageIdx(C0, C1)  # idx = s0 + s1*page  (page = bin k)
    _d = Src0 - _pg

    def _ref(in0, in1, s0, s1, imm2):
        P, S, N = in0.shape
        idx = (s0 + s1 * np.arange(S)).reshape(1, S, 1)
        hat = np.maximum(1.0 - np.abs(in0.astype(np.float32) - idx), 0.0)
        body = (hat * in1).astype(np.float32)
        return body, body.reshape(P, -1).sum(-1, keepdims=True)

    spec = Spec(
        body=relu(One - maxx(_d, Zero - _d)) * Src1,
        accum=_operator_add,
        accum_init=Zero,
        reference=_ref,
    )
    shas = {}
    for ver in ("v3", "v4"):
        uops = lower(spec, ver=ver)
        shas[ver] = DveOpSpec(name=name, opcode=1, uops=uops, rd1_en=True).sha(ver)
    op = dve_ops.DveOp(name, spec, subdim=True, uops_sha=shas)
    row = dve_ops._CUSTOM_DVE_ROW_BASE + len(dve_ops.OPS)
    assert row < 0x20, "custom DVE opcode rows exhausted"
    dve_ops.OPS.append(op)
    dve_ops.CUSTOM_DVE_SPECS[name] = op.spec
    dve_ops._SUB_OPCODE_FOR_NAME[name] = row
    return op


def _patch_act_tables():
    """Force Exp and Ln to resolve to the one table set containing both.

    The act-table-load pass assigns each activation the first set containing
    its function; Exp->exp_and_others and Ln->natural_log would otherwise
    alternate table loads (~1.3us each) on every exp->ln transition. Removing
    the two functions from every other set (list order and ids preserved)
    makes natural_log_exp_and_others serve both: one load for the kernel.
    """
    import concourse.bacc as bacc
    import concourse.hw_specs as hw_specs
    import concourse.mybir as mybir

    if getattr(_patch_act_tables, "_done", False):
        return
    orig = hw_specs.get_activation_tables
    Exp = mybir.ActivationFunctionType.Exp
    Ln = mybir.ActivationFunctionType.Ln

    def patched(module_arch):
        t = orig(module_arch)
        both = t.get("natural_log_exp_and_others")
        if both is not None and Exp in both and Ln in both:
            for name, fns in t.items():
                if name != "natural_log_exp_and_others":
                    fns.discard(Exp)
                    fns.discard(Ln)
        return t

    hw_specs.get_activation_tables = patched
    bacc.get_activation_tables = patched
    _patch_act_tables._done = True


def _build_nc():
    import concourse.bacc as bacc
    import concourse.mybir as mybir
    from concourse.tile import TileContext

    _patch_act_tables()
    hat_op = _register_hat_op()
    f32 = mybir.dt.float32
    bf16 = mybir.dt.bfloat16
    fp8 = mybir.dt.float8e4
    u8 = mybir.dt.uint8

    nc = bacc.Bacc("TRN2", target_bir_lowering=False, debug=False)
    # px-major logits, repacked so each (partition, unit) row is one
    # 1600B contiguous DRAM run (fp8 descriptors >= 512B keep full DMA bw)
    xp = nc.dram_tensor("xp", [BPC, 128, NJ, BINS, PX], fp8, kind="ExternalInput")
    # ch-major logits: partition p = ch*4 + strip, free = pixel in strip
    xc = nc.dram_tensor("xc", [BPC, 128, SPX], fp8, kind="ExternalInput")
    # slot 0 = mask (bf16 0/1), slots 1..4 = targets j-major
    tg = nc.dram_tensor("tg", [BPC, 128, NJ + 1, PX], bf16, kind="ExternalInput")
    # mask, lse-layout, replicated over j to match the psum column order:
    # mk2[p][b][16g+4j+s] = mask[b, s*6400 + g*128 + p]
    mk2 = nc.dram_tensor("mk2", [128, BPC, NG * 16], u8, kind="ExternalInput")
    # selector for bin-sum matmuls: sel[ch*4+s, j*4+s'] = (ch//8==j)&(s==s')
    sel = nc.dram_tensor("sel", [128, 16], bf16, kind="ExternalInput")
    acc_out = nc.dram_tensor("acc", [128, NACC], f32, kind="ExternalOutput")

    Exp = mybir.ActivationFunctionType.Exp
    Ln = mybir.ActivationFunctionType.Ln
    Alu = mybir.AluOpType
    HSPX = SPX // 2  # 3200: exp runs in strip-halves
    HB = BINS // 2

    with TileContext(nc) as tc:
        with (
            tc.tile_pool(name="pL", bufs=8) as pL,
            tc.tile_pool(name="pLc", bufs=2) as pLc,
            tc.tile_pool(name="pE", bufs=2) as pE,
            tc.tile_pool(name="pScr", bufs=3) as pScr,
            tc.tile_pool(name="pLse", bufs=3) as pLse,
            tc.tile_pool(name="pJ", bufs=5) as pJ,
            tc.tile_pool(name="pT", bufs=4) as pT,
            tc.tile_pool(name="pOnce", bufs=1) as pOnce,
            tc.tile_pool(name="psum", bufs=2, space="PSUM") as psum,
        ):
            accs = pOnce.tile([128, NACC], f32)
            nc.gpsimd.memset(accs[:, :], 0.0)
            sel_t = pOnce.tile([128, 16], bf16)
            m2_r = pOnce.tile([128, BPC, NG * 16], u8)

            # ---- head DMAs: first hat's data, then the lse stream, each
            # first tile split so its consumer starts as early as possible.
            t_raw0 = pT.tile([128, NJ + 1, PX], bf16, tag="t_raw")
            L00 = pL.tile([128, BINS, PX], fp8, tag="L")
            Lc0 = pLc.tile([128, SPX], fp8, tag="Lc")
            nc.sync.dma_start(t_raw0[:, 0:4, :], tg[0, :, 0:4, :])
            nc.sync.dma_start(L00[:, :, :], xp[0, :, 0])
            nc.sync.dma_start(Lc0[:, :HSPX], xc[0, :, :HSPX])
            nc.sync.dma_start(t_raw0[:, 4:, :], tg[0, :, 4:, :])
            L01 = pL.tile([128, BINS, PX], fp8, tag="L")
            nc.sync.dma_start(L01[:, :, :], xp[0, :, 1])
            nc.sync.dma_start(Lc0[:, HSPX:], xc[0, :, HSPX:])

            t_raws = [t_raw0]
            t2s, Es = [], []

            mf100s = {}

            def prep(b, t2, js):
                # t''[j] = 100*mask + t[j]  (slot 0 = mask, slot 1+j = t_j)
                j0, j1 = js[0], js[-1] + 1
                with tc.high_priority():
                    if b not in mf100s:
                        mf100 = pT.tile([128, PX], f32, tag="mf100")
                        nc.gpsimd.tensor_scalar(
                            out=mf100[:, :],
                            in0=t_raws[b][:, 0, :],
                            scalar1=100.0,
                            scalar2=None,
                            op0=Alu.mult,
                        )
                        mf100s[b] = mf100
                    nc.gpsimd.tensor_tensor(
                        out=t2[:, j0:j1, :],
                        in0=t_raws[b][:, 1 + j0 : 1 + j1, :],
                        in1=mf100s[b][:, :]
                        .unsqueeze(1)
                        .broadcast_to((128, j1 - j0, PX)),
                        op=Alu.add,
                    )

            # batch-0 j0..j2 prep: one DVE stt (Pool would gate the hats)
            t2_0 = pT.tile([128, NJ, PX], f32, tag="t2")
            t2s.append(t2_0)
            nc.vector.scalar_tensor_tensor(
                out=t2_0[:, 0:3, :],
                in0=t_raw0[:, 0, :].unsqueeze(1).broadcast_to((128, 3, PX)),
                scalar=100.0,
                in1=t_raw0[:, 1:4, :],
                op0=Alu.mult,
                op1=Alu.add,
            )

            def hat(b, j, L_ap, s0, col, nb=BINS):
                scr = pScr.tile([128, nb, PX], bf16, tag="scr")
                nc.vector._custom_dve(
                    hat_op,
                    out=scr[:, :, :],
                    in0=t2s[b][:, j, :].unsqueeze(1).broadcast_to((128, nb, PX)),
                    in1=L_ap,
                    s0=s0,
                    s1=1.0,
                    accum_out=accs[:, col : col + 1],
                )

            hat(0, 0, L00[:, :, :], 100.0, 0)

            E0 = pE.tile([128, SPX], bf16, tag="E")
            Es.append(E0)
            for h in range(2):
                nc.scalar.activation(
                    E0[:, h * HSPX : (h + 1) * HSPX],
                    Lc0[:, h * HSPX : (h + 1) * HSPX],
                    Exp,
                )

            def emit_npos(b):
                # acc[0, 25+b] = npos(b): Pool reduces over all axes (XYZWC)
                nc.gpsimd.tensor_reduce(
                    out=accs[0:1, 25 + b : 26 + b],
                    in_=t_raws[b][:, 0, :],
                    axis=mybir.AxisListType.XYZWC,
                    op=Alu.add,
                )

            def mm_half(b, ps, h, base=0):
                # bin sums via PE: E-slice stationary, selector moving
                # psum[p_px, 16 g + 4 j + s] = sum_k E[(8j+k)*4+s, 128g+p]
                for g in range(h * (NG // 2), (h + 1) * (NG // 2)):
                    nc.tensor.matmul(
                        out=ps[:, 16 * g - base : 16 * g + 16 - base],
                        lhsT=Es[b][:, 128 * g : 128 * g + 128],
                        rhs=sel_t[:, :],
                        start=True,
                        stop=True,
                    )

            HC = NG * 16 // 2  # 400
            HG = NG // 2

            def ln_mask_half(b, ps, lse, h, on_dve):
                nc.scalar.activation(
                    lse[:, h * HC : (h + 1) * HC],
                    ps[:, :],
                    Ln,
                )
                junk = pJ.tile([128, HC], bf16, tag="junk")
                if on_dve:
                    nc.vector.scalar_tensor_tensor(
                        out=junk[:, :],
                        in0=lse[:, h * HC : (h + 1) * HC],
                        scalar=1.0,
                        in1=m2_r[:, b, h * HC : (h + 1) * HC],
                        op0=Alu.mult,
                        op1=Alu.mult,
                        accum_out=accs[:, 17 + 2 * b + h : 18 + 2 * b + h],
                    )
                else:
                    nc.gpsimd.tensor_tensor(
                        out=junk[:, :],
                        in0=lse[:, h * HC : (h + 1) * HC],
                        in1=m2_r[:, b, h * HC : (h + 1) * HC],
                        op=Alu.mult,
                    )
                    nc.gpsimd.tensor_reduce(
                        out=accs[0:1, 17 + 2 * b + h : 18 + 2 * b + h],
                        in_=junk[:, :],
                        axis=mybir.AxisListType.XYZWC,
                        op=Alu.add,
                    )

            def lse_tail(b, on_dve=False):
                ps = psum.tile([128, NG * 16], f32, tag="ps")
                lse = pLse.tile([128, NG * 16], bf16, tag="lse")
                if b == BPC - 2:
                    for h in range(2):
                        psh = psum.tile([128, HC], f32, tag="psh")
                        mm_half(b, psh, h, base=h * HC)
                        ln_mask_half(b, psh, lse, h, False)
                    return
                if on_dve:
                    for h in range(2):
                        psh = psum.tile([128, HC], f32, tag="psh")
                        mm_half(b, psh, h, base=h * HC)
                        nc.scalar.activation(
                            lse[:, h * HC : (h + 1) * HC],
                            psh[:, :],
                            Ln,
                        )
                    junk = pJ.tile([128, NG * 16], bf16, tag="junk")
                    nc.vector.scalar_tensor_tensor(
                        out=junk[:, :],
                        in0=lse[:, :],
                        scalar=1.0,
                        in1=m2_r[:, b, :],
                        op0=Alu.mult,
                        op1=Alu.mult,
                        accum_out=accs[:, 17 + 2 * b : 18 + 2 * b],
                    )
                    return
                for h in range(2):
                    mm_half(b, ps, h)
                nc.scalar.activation(lse[:, :], ps[:, :], Ln)
                junk = pJ.tile([128, NG * 16], bf16, tag="junk")
                nc.gpsimd.tensor_tensor(
                    out=junk[:, :],
                    in0=lse[:, :],
                    in1=m2_r[:, b, :],
                    op=Alu.mult,
                )
                nc.gpsimd.tensor_reduce(
                    out=accs[0:1, 17 + 2 * b : 18 + 2 * b],
                    in_=junk[:, :],
                    axis=mybir.AxisListType.XYZWC,
                    op=Alu.add,
                )

            for b in range(1, BPC):
                # interleave: this batch's DMAs/compute + previous lse tail
                t_raw = pT.tile([128, NJ + 1, PX], bf16, tag="t_raw")
                t_raws.append(t_raw)
                t2 = pT.tile([128, NJ, PX], f32, tag="t2")
                t2s.append(t2)
                Lc = pLc.tile([128, SPX], fp8, tag="Lc")
                E = pE.tile([128, SPX], bf16, tag="E")
                Es.append(E)

                if b == 1:
                    # units (0,1..3) + next-batch staples
                    hat(0, 1, L01[:, :, :], 100.0, 1)
                    L02 = pL.tile([128, BINS, PX], fp8, tag="L")
                    nc.sync.dma_start(L02[:, :, :], xp[0, :, 2])
                    hat(0, 2, L02[:, :, :], 100.0, 2)
                    nc.sync.dma_start(t_raw[:, :, :], tg[b])
                    prep(b, t2, (0, 1, 2, 3))
                    emit_npos(0)
                    L03 = pL.tile([128, BINS, PX], fp8, tag="L")
                    nc.sync.dma_start(L03[:, :, :], xp[0, :, 3])
                    hat(0, 3, L03[:, :, :], 100.0, 3)
                else:
                    nc.sync.dma_start(t_raw[:, :, :], tg[b])
                    prep(b, t2, (0, 1, 2, 3))
                    emit_npos(b - 1)

                nc.sync.dma_start(Lc[:, :HSPX], xc[b, :, :HSPX])
                nc.scalar.activation(E[:, :HSPX], Lc[:, :HSPX], Exp)
                if b == 1:
                    nc.sync.dma_start(sel_t[:, :], sel[:, :])
                if b == BPC - 1:
                    # emit batch-2's ln+mask chain between exp3's halves:
                    # ln3's end time is unchanged (ScalarE work conserved)
                    # but Pool's serial tt+reduce chain starts ~2.9us sooner
                    lse_tail(b - 1, on_dve=False)
                for j in range(NJ):
                    L = pL.tile([128, BINS, PX], fp8, tag="L")
                    nc.sync.dma_start(L[:, :, :], xp[b, :, j])
                    hat(b, j, L[:, :, :], 100.0, b * NJ + j)
                    if j == 0:
                        nc.sync.dma_start(Lc[:, HSPX:], xc[b, :, HSPX:])
                        nc.scalar.activation(E[:, HSPX:], Lc[:, HSPX:], Exp)
                    if j == 1 and b == 1:
                        nc.sync.dma_start(m2_r[:, :, :], mk2[:, :, :])
                if b != BPC - 1:
                    lse_tail(b - 1, on_dve=False)

            emit_npos(BPC - 1)
            lse_tail(BPC - 1, on_dve=True)

            nc.sync.dma_start(acc_out[:, :], accs[:, :])

    nc.finalize()
    return nc


def _get_nc():
    with _lock:
        if "nc" not in _cache:
            _cache["nc"] = _build_nc()
        return _cache["nc"]


def kernel(reg_logits: np.ndarray, targets: np.ndarray, pos_mask: np.ndarray) -> np.ndarray:
    import concourse.mybir as mybir
    from concourse.bass_utils import run_bass_kernel_spmd

    nc = _get_nc()
    fp8_np = mybir.dt.np(mybir.dt.float8e4)
    bf16_np = mybir.dt.np(mybir.dt.bfloat16)

    lg = np.ascontiguousarray(reg_logits, dtype=np.float32).reshape(B, C, HW)
    lg8 = lg.astype(fp8_np)
    # slot 0 = mask (bf16), slots 1..4 = targets j-major: [b, blk, 5, px]
    mask_u8 = np.ascontiguousarray(pos_mask).astype(np.uint8)
    tg16 = np.empty((B, 128, NJ + 1, PX), dtype=bf16_np)
    tg16[:, :, 0, :] = mask_u8.reshape(B, 128, PX)
    tg16[:, :, 1:, :] = (
        np.ascontiguousarray(targets, dtype=np.float32)
        .astype(bf16_np)
        .reshape(B, 128, PX, NJ)
        .transpose(0, 1, 3, 2)
    )

    # selector: sel[ch*4+s, j*4+s'] = 1 if ch//8 == j and s == s'
    sel = np.zeros((128, 16), dtype=bf16_np)
    for ch in range(C):
        for s in range(NSTRIP):
            sel[ch * NSTRIP + s, (ch // BINS) * NSTRIP + s] = 1.0

    in_maps = []
    for c in range(NCORES):
        b0 = c * BPC
        # px-major: [b, blk, j, c8, px] from [c, hw]
        xp_core = (
            lg8[b0 : b0 + BPC]
            .reshape(BPC, NJ, BINS, 128, PX)
            .transpose(0, 3, 1, 2, 4)
        )
        # ch-major: [b, ch*4+s, strip-px]
        xc_core = lg8[b0 : b0 + BPC].reshape(BPC, C * NSTRIP, SPX)
        # lse-layout mask, j-replicated: [p, b, 16g+4j+s]
        mk2_core = np.broadcast_to(
            mask_u8[b0 : b0 + BPC]
            .reshape(BPC, NSTRIP, NG, 128)
            .transpose(3, 0, 2, 1)[:, :, :, None, :],  # p b g j s
            (128, BPC, NG, NJ, NSTRIP),
        ).reshape(128, BPC, NG * 16)
        in_maps.append(
            {
                "xp": np.ascontiguousarray(xp_core),
                "xc": np.ascontiguousarray(xc_core),
                "tg": tg16[b0 : b0 + BPC],
                "mk2": np.ascontiguousarray(mk2_core),
                "sel": sel,
            }
        )

    res = run_bass_kernel_spmd(nc, in_maps, core_ids=list(range(NCORES)))

    tot_interp = 0.0
    tot_lse = 0.0
    npos100 = 0.0
    for r in res.results:
        a = r["acc"].astype(np.float64)
        tot_interp += a[:, :17].sum()
        tot_lse += a[:, 17:25].sum()
        npos100 += a[:, 25:29].sum()

    npos = npos100
    total = tot_lse - tot_interp
    loss = total / (max(npos, 1.0) * 4.0) if npos > 0 else 0.0
    return np.float32(loss)


if __name__ == "__main__":
    rng = np.random.default_rng(0)
    rl = rng.standard_normal((B, C, H, W), dtype=np.float32)
    tg = (rng.random((B, HW, NJ), dtype=np.float32) * (BINS - 1)).astype(np.float32)
    pm = rng.integers(0, 2, size=(B, HW)).astype(bool)
    print(kernel(reg_logits=rl, targets=tg, pos_mask=pm))
